# revision 59
# baseline (speedup 1.0000x reference)
"""Trainium2 Bass kernel for multi-head attention (B=4, N=2048, H=1024, 16 heads).

Sharding: 8 cores = 4 batches x 2 head-groups (8 heads each).  Each core:
  - computes q/k/v projections for its 8 heads from its batch's x,
  - applies RoPE, runs SDPA (scores kept transposed: [kv, q]-layout so the
    softmax denominator comes from an extra ones-column in the PV matmul),
  - computes the partial output projection over its 512 head-dim columns.
Host sums the two partials per batch.

The SDPA is paced by the exp stream on the Activation engine (~262k
elements/partition, the hard per-core floor); everything else hides under
it.  PV runs "flipped" (q on PSUM partitions, head-dim on the free axis):
each [128q, 65] accumulator costs the PE only 65 cycles per kv block
instead of streaming 512 q columns per head, the softmax denominator
lands per-partition (normalized by a DVE tensor_scalar multiply — no
cross-partition broadcast needed), and a cheap PE transpose restores the
[c, token] layout the output projection wants.  PSUM start=True zeroes a
whole bank, so the four q-subtile accumulation regions sharing a bank are
pre-zeroed with a DVE memset and all PV matmuls accumulate (start=False).
All non-SDPA work — projections + RoPE, the previous q-chunk's
normalize/transpose epilogue, the next pair's q/k/v, finished
out-projection token blocks — is emitted as deadline-aware fillers inside
the kv loops so neither PE nor ACT stalls at chunk boundaries.

All matmuls run in bf16 with fp32 PSUM accumulation.
"""

import numpy as np
import ml_dtypes
from functools import partial

B, N, H, HEADS, D = 4, 2048, 1024, 16, 64
N_CORES = 8
HPC = 8          # heads per core
PAIRS = HPC // 2

_BF16 = ml_dtypes.bfloat16

_NC_CACHE = {}

# adjacent-pair swap for rotate-half on interleaved head dims
_PAIR_SWAP_MASK = [i ^ 1 for i in range(32)]

# head-dim interleave [d0, d32, d1, d33, ...] so rotate-half partners are
# adjacent partitions
_PERM = np.empty(64, dtype=np.int64)
_PERM[0::2] = np.arange(32)
_PERM[1::2] = 32 + np.arange(32)
# sign of the sin term per interleaved row: even rows take -sin (partner is
# the upper half), odd rows take +sin
_SIGN = np.where(np.arange(64) % 2 == 0, -1.0, 1.0).astype(np.float32)


def build_bass(n_tokens=N, reps=1):
    """Build the per-core Bass module (same program on all 8 cores).

    reps > 1 emits the whole body that many times (same inputs, same
    output) — used only for steady-state timing, where the marginal cost
    per extra rep isolates device execution time from the axon tunnel's
    fixed dispatch latency."""
    from contextlib import ExitStack
    import concourse.bass as bass
    import concourse.mybir as mybir
    import concourse.tile as tile
    from concourse import bacc
    from concourse.masks import make_identity

    f32 = mybir.dt.float32
    bf16 = mybir.dt.bfloat16
    Exp = mybir.ActivationFunctionType.Exp

    NT = n_tokens
    TB = NT // 128        # 128-token blocks
    TC = NT // 512        # 512-token chunks
    KV = TB
    QC = TC

    nc = bacc.Bacc()

    x_bf = nc.dram_tensor("x_bf", [128, 8, NT], bf16, kind="ExternalInput")
    wqkv = nc.dram_tensor("wqkv", [128, 8, 3 * 512], bf16, kind="ExternalInput")
    wo = nc.dram_tensor("wo", [128, 4, H], bf16, kind="ExternalInput")
    cos_r = nc.dram_tensor("cos_r", [128, NT], f32, kind="ExternalInput")
    sin_r = nc.dram_tensor("sin_r", [128, NT], f32, kind="ExternalInput")
    y = nc.dram_tensor("y", [NT, H], f32, kind="ExternalOutput")

    with ExitStack() as ctx:
        tc_ = ctx.enter_context(tile.TileContext(nc))

        sing = ctx.enter_context(tc_.tile_pool(name="resident", bufs=1))
        qkv_ps = ctx.enter_context(tc_.tile_pool(name="qkv_ps", bufs=2, space="PSUM"))
        sc_ps = ctx.enter_context(tc_.tile_pool(name="sc_ps", bufs=2, space="PSUM"))
        pv_ps = ctx.enter_context(tc_.tile_pool(name="pv_ps", bufs=2, space="PSUM"))
        rope_sb = ctx.enter_context(tc_.tile_pool(name="rope_sb", bufs=3))
        p_pool = ctx.enter_context(tc_.tile_pool(name="p_pool", bufs=8))
        norm_sb = ctx.enter_context(tc_.tile_pool(name="norm_sb", bufs=3))
        y_pool = ctx.enter_context(tc_.tile_pool(name="y_pool", bufs=2))

        # identity for PE transposes (shared across reps)
        ident = sing.tile([128, 128], bf16, name="ident", tag="ident")
        make_identity(nc, ident)

        # preload the exp table set while the input DMAs run (first use of a
        # table set costs ~2.7us on ScalarE)
        warm = norm_sb.tile([1, 1], f32, name="warm", tag="warm", bufs=1)
        nc.vector.memset(warm, 0.0)
        nc.scalar.activation(warm, warm, Exp)

        for _ in range(reps):
            _emit_body(nc, tile, mybir, sing, qkv_ps, sc_ps, pv_ps, rope_sb,
                       p_pool, norm_sb, y_pool, ident,
                       x_bf, wqkv, wo, cos_r, sin_r, y, NT, TB, TC, KV, QC)

    nc.finalize()
    return nc


def _emit_body(nc, tile, mybir, sing, qkv_ps, sc_ps, pv_ps, rope_sb, p_pool,
               norm_sb, y_pool, ident, x_bf, wqkv, wo, cos_r, sin_r, y,
               NT, TB, TC, KV, QC):
    f32 = mybir.dt.float32
    bf16 = mybir.dt.bfloat16
    Exp = mybir.ActivationFunctionType.Exp

    # -------- resident SBUF tensors --------
    xT = sing.tile([128, 8, NT], bf16, name="xT", tag="xT")
    wqkv_sb = sing.tile([128, 8, 3 * 512], bf16, name="wqkv_sb", tag="wqkv_sb")
    wo_sb = sing.tile([128, 4, H], bf16, name="wo_sb", tag="wo_sb")
    cos_sb = sing.tile([128, NT], f32, name="cos_sb", tag="cos_sb")
    sin_sb = sing.tile([128, NT], f32, name="sin_sb", tag="sin_sb")
    qT = [sing.tile([128, NT], bf16, name=f"qT{p}", tag=f"qT{p}") for p in range(PAIRS)]
    kT = [sing.tile([128, NT], bf16, name=f"kT{p}", tag=f"kT{p}") for p in range(PAIRS)]
    # v_aug[p][part, kvblock, head, 65]: col 0 = ones (softmax denominator),
    # cols 1..64 = v head dims
    vaug = [sing.tile([128, KV, 2, 65], bf16, name=f"vaug{p}", tag=f"vaug{p}")
            for p in range(PAIRS)]
    # attnT[p]: [c, token] layout (c = pair p's 128 head-dim columns),
    # normalized attention output, feeds the output projection as lhsT
    attnT = [sing.tile([128, NT], bf16, name=f"attnT{p}", tag=f"attnT{p}")
             for p in range(PAIRS)]

    def qk_parts(p, off, dst, tcn):
        """q or k projection for one head pair and one 512-token chunk,
        followed by RoPE into dst — split into two ~0.85us filler halves
        so a whole 1.7us group never delays the next scores emission on
        the in-order PE."""
        tsl = slice(tcn * 512, (tcn + 1) * 512)
        cell = {}

        def part_a():
            ps = qkv_ps.tile([128, 512], f32, name="ps_qk", tag="qkvps")
            for kc in range(4):
                nc.tensor.matmul(
                    ps,
                    lhsT=wqkv_sb[:, kc, off + p * 128: off + (p + 1) * 128],
                    rhs=xT[:, kc, tsl],
                    start=(kc == 0), stop=False,
                )
            cell["ps"] = ps

        def part_b():
            ps = cell["ps"]
            for kc in range(4, 8):
                nc.tensor.matmul(
                    ps,
                    lhsT=wqkv_sb[:, kc, off + p * 128: off + (p + 1) * 128],
                    rhs=xT[:, kc, tsl],
                    start=False, stop=(kc == 7),
                )
            # RoPE: out = ps * cos + rot(ps) * sin_signed.
            # Head dims are host-interleaved [d0,d32,d1,d33,...], so
            # rotate-half is an adjacent-pair swap: a single DVE
            # stream_shuffle (within-quadrant permutation).
            rot = rope_sb.tile([128, 512], f32, name="rot", tag="rot")
            nc.vector.stream_shuffle(rot, ps, mask=_PAIR_SWAP_MASK)
            t1 = rope_sb.tile([128, 512], f32, name="t1", tag="t1")
            nc.vector.tensor_mul(t1, ps, cos_sb[:, tsl])
            t2 = rope_sb.tile([128, 512], f32, name="t2", tag="t2")
            nc.gpsimd.tensor_mul(t2, rot, sin_sb[:, tsl])
            nc.gpsimd.tensor_add(dst[:, tsl], t1, t2)

        return part_a, part_b

    def qk_group(p, off, dst, tcn):
        a, b = qk_parts(p, off, dst, tcn)
        a()
        b()

    # -------- input loads: k-projection inputs first so SDPA can start
    # as early as possible.  Two HWDGE rings: weights/rope tables on the
    # scalar ring, x on the sync ring.
    nc.scalar.dma_start(out=wqkv_sb[:, :, 512:1024], in_=wqkv[:, :, 512:1024])
    # rope tables in halves so the first k/q chunks' rope starts sooner;
    # q and v weights ahead of the second halves (v gates the first PV)
    half = NT // 2
    nc.scalar.dma_start(out=cos_sb[:, 0:half], in_=cos_r[:, 0:half])
    nc.scalar.dma_start(out=sin_sb[:, 0:half], in_=sin_r[:, 0:half])
    nc.scalar.dma_start(out=wqkv_sb[:, :, 0:512], in_=wqkv[:, :, 0:512])
    nc.scalar.dma_start(out=wqkv_sb[:, :, 1024:1536], in_=wqkv[:, :, 1024:1536])
    nc.scalar.dma_start(out=cos_sb[:, half:NT], in_=cos_r[:, half:NT])
    nc.scalar.dma_start(out=sin_sb[:, half:NT], in_=sin_r[:, half:NT])
    nc.scalar.dma_start(out=wo_sb, in_=wo[:, :, :])
    for p in range(PAIRS):
        nc.vector.memset(vaug[p][:, :, :, 0:1], 1.0)

    # x is host-transposed: plain contiguous loads, one per 512-token
    # chunk, on the sync ring; first chunk split so k-proj starts sooner
    nc.sync.dma_start(out=xT[:, :, 0:256], in_=x_bf[:, :, 0:256])
    nc.sync.dma_start(out=xT[:, :, 256:512], in_=x_bf[:, :, 256:512])
    for tcn in range(1, TC):
        tsl = slice(tcn * 512, (tcn + 1) * 512)
        nc.sync.dma_start(out=xT[:, :, tsl], in_=x_bf[:, :, tsl])

    def v_group(p, tb):
        """v projection for one head pair and one 128-token block, into
        the augmented-v layout (col 0 stays the ones column)."""
        ps_v = qkv_ps.tile([128, 512], f32, name="ps_v", tag="qkvps")
        for kc in range(8):
            nc.tensor.matmul(
                ps_v[:, 0:128],
                lhsT=xT[:, kc, tb * 128:(tb + 1) * 128],
                rhs=wqkv_sb[:, kc, 1024 + p * 128:1024 + (p + 1) * 128],
                start=(kc == 0), stop=(kc == 7),
            )
        ps_v2 = ps_v[:, 0:128].rearrange("a (h d) -> a h d", h=2)
        nc.vector.tensor_copy(out=vaug[p][:, tb, :, 1:65], in_=ps_v2)

    # k/q projections for pair 0, first chunks only: the minimum needed to
    # start SDPA (its first 4 kv blocks touch only k chunk 0).  Everything
    # else (k chunks 1-3, later q chunks, v, later pairs) interleaves into
    # the SDPA loops as fillers, paced to the DMA arrival of x.
    qk_group(0, 512, kT[0], 0)
    qk_group(0, 0, qT[0], 0)

    def emit_normalize(pend):
        """Epilogue part 1 for a finished (pair, qc): per-token reciprocal
        of the denominator, then scale the 64 context columns (DVE)."""
        pp, pqc, po4, st = pend
        rc4 = norm_sb.tile([128, 2, 4], f32, name="rc4", tag="rc4")
        for hh in range(2):
            nc.vector.reciprocal(rc4[:, hh, :], po4[hh][:, :, 0])
        nstg = norm_sb.tile([128, 4, 2, 64], bf16, name="nstg", tag="nstg")
        for hh in range(2):
            for qs in range(4):
                nc.vector.tensor_scalar_mul(
                    nstg[:, qs, hh, :],
                    po4[hh][:, qs, 1:65],
                    rc4[:, hh, qs:qs + 1],
                )
        st["nstg"] = nstg

    def emit_transposes(pend):
        """Epilogue part 2: [token, c] -> [c, token] PE transposes for the
        out projection.  PSUM target reuses the already-read o4 region."""
        pp, pqc, po4, st = pend
        nstg = st["nstg"]
        for qs in range(4):
            tview = po4[0][:, qs, 1:65].bitcast(bf16)
            nc.tensor.transpose(tview, nstg[:, qs, :, :], ident)
            tok = pqc * 512 + qs * 128
            nc.vector.tensor_copy(out=attnT[pp][:, tok:tok + 128],
                                  in_=tview)

    def emit_outproj_oc(tb, oc):
        """Half an output-projection token block (one 512-column slice) —
        filler-sized so it never delays the next scores emission."""
        tsl = slice(tb * 128, (tb + 1) * 128)
        osl = slice(oc * 512, (oc + 1) * 512)
        ps_y = qkv_ps.tile([128, 512], f32, name="ps_y", tag="qkvps")
        for pp in range(PAIRS):
            nc.tensor.matmul(
                ps_y,
                lhsT=attnT[pp][:, tsl],
                rhs=wo_sb[:, pp, osl],
                start=(pp == 0), stop=(pp == PAIRS - 1),
            )
        ysb = y_pool.tile([128, 512], f32, name="ysb", tag="ysb", bufs=4)
        nc.vector.tensor_copy(out=ysb, in_=ps_y)
        nc.sync.dma_start(out=y[tsl, osl], in_=ysb)

    def emit_outproj(tb):
        emit_outproj_oc(tb, 0)
        emit_outproj_oc(tb, 1)

    # -------- SDPA with deadline-aware fillers --------
    # The exp stream on ACT paces SDPA; the PE has slack under it.  All
    # remaining work (previous qc's normalize/transpose epilogue, this
    # pair's later q chunks, the next pair's q/k/v projections, finished
    # out-projection blocks) is emitted as "fillers", at most ~one per kv
    # iteration.  EMISSION ORDER IS CORRECTNESS: the tile framework only
    # syncs readers against earlier writers, so every filler must be
    # emitted before its first consumer; fillers are force-drained at the
    # end of each qc.
    pending = None
    outproj_done = 0
    # one-step scores lookahead: the next qc's first scores+exp are emitted
    # BEFORE the current qc's final PV step (which waits on the last exp),
    # so ACT never idles across a qc boundary waiting for the in-order PE
    # to reach the next chunk's scores
    hoisted = None

    def emit_sc(p, qc, kv):
        qsl = slice(qc * 512, (qc + 1) * 512)
        ksl = slice(kv * 128, (kv + 1) * 128)
        sc = sc_ps.tile([128, 2, 512], f32, name="sc", tag="sc")
        nc.tensor.matmul(sc[:, 0, :], lhsT=kT[p][0:64, ksl],
                         rhs=qT[p][0:64, qsl])
        nc.tensor.matmul(sc[:, 1, :], lhsT=kT[p][64:128, ksl],
                         rhs=qT[p][64:128, qsl])
        pt = p_pool.tile([128, 2, 512], bf16, name="pt", tag="pt")
        nc.scalar.activation(pt, sc, Exp)
        return pt

    for p in range(PAIRS):
        for qc in range(QC):
            qsl = slice(qc * 512, (qc + 1) * 512)
            # PV accumulators, q on partitions: o4[hh][:, qs, 0] is the
            # softmax denominator (ones column of vaug), cols 1..64 the
            # unnormalized context.  One full PSUM bank each (padded); the
            # explicit memset replaces start=True (which would zero the
            # whole bank and wipe the sibling qs accumulation regions).
            o4 = [pv_ps.tile([128, 4, 65], f32, name=f"o4_{hh}", tag="pv",
                             padded_shape=[128, 4, 128])
                  for hh in range(2)]
            for hh in range(2):
                nc.vector.memset(o4[hh], 0.0)

            fillers = []
            if p == 0 and qc == 0:
                # k chunks 1-3: chunk j gates the scores at kv = 4j, and
                # x chunk j arrives ~3us after chunk j-1 — filler pacing
                # (one per kv step) matches both
                for tcn in range(1, TC):
                    fillers.extend(qk_parts(0, 512, kT[0], tcn))
            if pending is not None:
                fillers.append(partial(emit_normalize, pending))
                fillers.append(partial(emit_transposes, pending))
                pending = None
            if qc + 1 < QC:
                # next q chunk of this pair (needed by the next qc)
                fillers.extend(qk_parts(p, 0, qT[p], qc + 1))
            if p + 1 < PAIRS:
                # next pair's projections: k (4 chunks) + first q chunk
                # spread over this pair's 4 qcs, v (16 blocks) over qcs
                # 1..3 (needed only by the next pair's first PV step)
                if qc < TC:
                    fillers.extend(qk_parts(p + 1, 512, kT[p + 1], qc))
                if qc == QC - 1:
                    fillers.extend(qk_parts(p + 1, 0, qT[p + 1], 0))
                if qc >= 1:
                    lo, hi = (qc - 1) * 6, min(TB, qc * 6)
                    for tb in range(lo, hi):
                        fillers.append(partial(v_group, p + 1, tb))
            else:
                # last pair: fold finished out-projection token blocks in
                if qc >= 1:
                    tb_hi = qc * 4
                    while outproj_done < tb_hi:
                        fillers.append(partial(emit_outproj_oc, outproj_done, 0))
                        fillers.append(partial(emit_outproj_oc, outproj_done, 1))
                        outproj_done += 1
            fillers.reverse()  # pop() from the front

            # software-pipelined: PV lags one kv step so the PE stream
            # issues the next scores before the exp-gated PV matmuls
            pt_prev = None

            def pv_step(kv, pt_tile):
                last = kv == KV - 1
                for hh in range(2):
                    for qs in range(4):
                        nc.tensor.matmul(
                            o4[hh][:, qs, :],
                            lhsT=pt_tile[:, hh, qs * 128:(qs + 1) * 128],
                            rhs=vaug[p][:, kv, hh, :],
                            start=False, stop=last,
                            skip_group_check=True,
                        )

            if hoisted is not None:
                pt_prev = hoisted
                hoisted = None
            else:
                pt_prev = emit_sc(p, qc, 0)
                if p == 0 and qc == 0:
                    v_group(0, 0)
            for kv in range(1, KV):
                pt = emit_sc(p, qc, kv)
                # filler/v work sits between the scores and the exp-gated
                # PV step, covering the ACT->PE semaphore latency window
                # so the in-order PE never runs dry waiting on the exp
                if p == 0 and qc == 0:
                    v_group(0, kv)
                if fillers:
                    fillers.pop()()
                pv_step(kv - 1, pt_prev)
                pt_prev = pt
            # lookahead: next chunk's first scores + exp ahead of the
            # exp-gated final PV step
            if not (p == PAIRS - 1 and qc == QC - 1):
                nqc = (qc + 1) % QC
                np_ = p + 1 if nqc == 0 else p
                hoisted = emit_sc(np_, nqc, 0)
            pv_step(KV - 1, pt_prev)
            while fillers:  # force-drain: consumers follow in the next qc
                fillers.pop()()
            pending = (p, qc, o4, {})

    # final drain, pipelined per q-subtile: each 128-token block's
    # normalize -> transpose -> out-projection chain overlaps the next
    # subtile's normalize on DVE
    pp, pqc, po4, _ = pending
    for qs in range(4):
        rc2 = norm_sb.tile([128, 2, 1], f32, name="rc2", tag="rc2")
        nstg4 = norm_sb.tile([128, 2, 64], bf16, name="nstg4", tag="nstg4")
        for hh in range(2):
            nc.vector.reciprocal(rc2[:, hh, :], po4[hh][:, qs, 0:1])
            nc.vector.tensor_scalar_mul(
                nstg4[:, hh, :], po4[hh][:, qs, 1:65], rc2[:, hh, 0:1])
        tview = po4[0][:, qs, 1:65].bitcast(bf16)
        nc.tensor.transpose(tview, nstg4, ident)
        tok = pqc * 512 + qs * 128
        nc.vector.tensor_copy(out=attnT[pp][:, tok:tok + 128], in_=tview)
        emit_outproj_oc(tok // 128, 0)
        emit_outproj_oc(tok // 128, 1)
    pending = None
    outproj_done = TB


def get_bass(n_tokens=N, reps=1):
    key = (n_tokens, reps)
    if key not in _NC_CACHE:
        _NC_CACHE[key] = build_bass(n_tokens, reps)
    return _NC_CACHE[key]


def host_prep(x, rotary_emb, w_qkv, w_out, n_tokens=N, n_batches=B):
    """Build the 8 per-core input maps from the full-size inputs."""
    x = np.asarray(x, dtype=np.float32)
    rotary_emb = np.asarray(rotary_emb, dtype=np.float32)
    w_qkv = np.asarray(w_qkv, dtype=np.float32)
    w_out = np.asarray(w_out, dtype=np.float32)

    x_bf = x.astype(_BF16)
    # pre-transposed per batch: [p, hc, t] = x[b, t, hc*128+p]
    x_t_dev = [np.ascontiguousarray(
        x_bf[b].T.reshape(8, 128, n_tokens).transpose(1, 0, 2))
        for b in range(n_batches)]

    # rope tables in the interleaved head-dim order (duplicated for the two
    # heads sharing a partition block)
    cos_t = np.cos(rotary_emb).T.astype(np.float32)[_PERM]     # [64, NT]
    sin_t = (np.sin(rotary_emb).T.astype(np.float32)[_PERM]) * _SIGN[:, None]
    cos128 = np.ascontiguousarray(np.concatenate([cos_t, cos_t], axis=0))
    sin128 = np.ascontiguousarray(np.concatenate([sin_t, sin_t], axis=0))

    # per-head column permutation of the q/k projection outputs
    qk_col_perm = (np.arange(8)[:, None] * 64 + _PERM[None, :]).reshape(-1)

    per_group = []
    for g in range(2):
        gs = slice(g * 512, (g + 1) * 512)
        wq_t = w_qkv[0 * H:1 * H][gs].T * (1.0 / np.sqrt(D))   # [1024, 512], scale folded
        wk_t = w_qkv[1 * H:2 * H][gs].T
        wv_t = w_qkv[2 * H:3 * H][gs].T
        wq_t = wq_t[:, qk_col_perm]
        wk_t = wk_t[:, qk_col_perm]
        wqkv_t = np.concatenate([wq_t, wk_t, wv_t], axis=1)    # [1024, 1536]
        wqkv_dev = np.ascontiguousarray(
            wqkv_t.reshape(8, 128, 3 * 512).transpose(1, 0, 2)).astype(_BF16)
        wo_t = w_out[:, gs].T                                   # [512, 1024]
        wo_dev = np.ascontiguousarray(
            wo_t.reshape(4, 128, H).transpose(1, 0, 2)).astype(_BF16)
        per_group.append((wqkv_dev, wo_dev))

    in_maps = []
    for c in range(2 * n_batches):
        b, g = c // 2, c % 2
        wqkv_dev, wo_dev = per_group[g]
        in_maps.append({
            "x_bf": x_t_dev[b],
            "wqkv": wqkv_dev,
            "wo": wo_dev,
            "cos_r": cos128,
            "sin_r": sin128,
        })
    return in_maps


def run_on_hw(in_maps, n_tokens=N, trace=False):
    from concourse.bass_utils import run_bass_kernel_spmd
    nc = get_bass(n_tokens)
    core_ids = list(range(len(in_maps)))
    try:
        return run_bass_kernel_spmd(nc, in_maps, core_ids, trace=trace)
    except ModuleNotFoundError:
        # axon NTFF profiling hook unavailable in this container
        return run_bass_kernel_spmd(nc, in_maps, core_ids, trace=False)


def make_sharded_callable(in_maps, n_tokens=N, donate=True, reps=1):
    """Replicates bass2jax.run_bass_via_pjrt's multi-core path but returns a
    reusable jitted callable + prepared host args, for steady-state timing."""
    import jax
    import numpy as _np
    from jax.sharding import Mesh, PartitionSpec, NamedSharding
    from jax.experimental.shard_map import shard_map
    import concourse.mybir as mybir
    from concourse import bass2jax

    bass2jax.install_neuronx_cc_hook()
    nc = get_bass(n_tokens, reps)
    n_cores = len(in_maps)

    partition_name = nc.partition_id_tensor.name if nc.partition_id_tensor else None
    in_names, out_names, out_avals, zero_outs = [], [], [], []
    for alloc in nc.m.functions[0].allocations:
        if not isinstance(alloc, mybir.MemoryLocationSet):
            continue
        name = alloc.memorylocations[0].name
        if alloc.kind == "ExternalInput":
            if name != partition_name:
                in_names.append(name)
        elif alloc.kind == "ExternalOutput":
            shape = tuple(alloc.tensor_shape)
            dtype = mybir.dt.np(alloc.dtype)
            out_names.append(name)
            out_avals.append(jax.core.ShapedArray(shape, dtype))
            zero_outs.append(_np.zeros(shape, dtype))
    n_params = len(in_names)
    n_outs = len(out_avals)
    all_in_names = list(in_names) + out_names
    if partition_name is not None:
        all_in_names.append(partition_name)

    def _body(*args):
        operands = list(args)
        if partition_name is not None:
            operands.append(bass2jax.partition_id_tensor())
        outs = bass2jax._bass_exec_p.bind(
            *operands,
            out_avals=tuple(out_avals),
            in_names=tuple(all_in_names),
            out_names=tuple(out_names),
            lowering_input_output_aliases=(),
            sim_require_finite=True,
            sim_require_nnan=True,
            nc=nc,
        )
        return tuple(outs)

    devices = jax.devices()[:n_cores]
    mesh = Mesh(_np.asarray(devices), ("core",))
    in_specs = (PartitionSpec("core"),) * (n_params + n_outs)
    out_specs = (PartitionSpec("core"),) * n_outs
    donate_idx = tuple(range(n_params, n_params + n_outs)) if donate else ()
    sharded = jax.jit(
        shard_map(_body, mesh=mesh, in_specs=in_specs, out_specs=out_specs,
                  check_rep=False),
        donate_argnums=donate_idx,
        keep_unused=True,
    )
    per_core = [[_np.asarray(m[name]) for name in in_names] for m in in_maps]
    concat_in = [
        _np.concatenate([per_core[c][i] for c in range(n_cores)], axis=0)
        for i in range(n_params)
    ]
    concat_zeros = [
        _np.zeros((n_cores * z.shape[0], *z.shape[1:]), z.dtype) for z in zero_outs
    ]
    sharding = NamedSharding(mesh, PartitionSpec("core"))
    return sharded, concat_in, concat_zeros, sharding, out_names, out_avals


def time_kernel(in_maps, n_tokens=N, iters=6):
    """Steady-state wall time of one sharded NEFF execution (device-resident
    inputs; measures dispatch + exec + sync)."""
    import time as _time
    import jax
    sharded, concat_in, concat_zeros, sharding, _, _ = make_sharded_callable(
        in_maps, n_tokens)
    times = []
    for _ in range(iters):
        args = [jax.device_put(a, sharding) for a in concat_in + concat_zeros]
        jax.block_until_ready(args)
        t0 = _time.perf_counter()
        outs = sharded(*args)
        jax.block_until_ready(outs)
        times.append(_time.perf_counter() - t0)
    return times


def time_kernel_steady(in_maps, n_tokens=N, batch=32, lo_reps=4,
                       timing_reps=12, meas=8):
    """Steady-state per-execution device time of the kernel.

    Without NTFF tracing, a single dispatch through the axon tunnel is
    dominated by ~70-110ms network latency plus ~0.7ms per-call dispatch
    overhead.  To isolate device time, build NEFFs whose body is the
    kernel repeated `lo_reps` / `timing_reps` times, time `batch`
    back-to-back dispatches of both variants, and take the slope:

        per_exec = (T(hi) - T(lo)) / (batch * (hi - lo))

    Both fixed latency and per-dispatch overhead cancel; what remains is
    the marginal device time for one extra execution of the full kernel
    (input DMA from HBM + compute + output DMA).  Using lo_reps > 1 puts
    both measurement points in the sustained-load regime, so they share
    thermal/network conditions.  Returns (per_exec_s, diag).
    """
    import time as _time
    import statistics
    import jax

    def make(reps):
        sharded, concat_in, concat_zeros, sharding, _, _ = make_sharded_callable(
            in_maps, n_tokens, donate=False, reps=reps)
        args = [jax.device_put(a, sharding) for a in concat_in + concat_zeros]
        jax.block_until_ready(args)
        outs = sharded(*args)
        jax.block_until_ready(outs)

        def run():
            t0 = _time.perf_counter()
            outs = None
            for _ in range(batch):
                outs = sharded(*args)
            jax.block_until_ready(outs)
            return _time.perf_counter() - t0
        return run

    run_lo = make(lo_reps)
    run_hi = make(timing_reps)
    # temporally-paired batches cancel slow network drift; alternating the
    # order within each pair cancels load-ramp asymmetry, and the median
    # of per-pair slopes is robust to latency bursts.  If the spread is
    # wide (noisy environment window), keep sampling up to 2x.
    pairs = []

    def add_pair(i):
        if i % 2 == 0:
            t_lo = run_lo()
            t_hi = run_hi()
        else:
            t_hi = run_hi()
            t_lo = run_lo()
        pairs.append((t_lo, t_hi))

    def slopes_of():
        return [(t_hi - t_lo) / (batch * (timing_reps - lo_reps))
                for t_lo, t_hi in pairs[1:]]  # drop warmup pair

    for i in range(meas + 1):
        add_pair(i)
    while len(pairs) < 2 * meas + 1:
        sl = sorted(slopes_of())
        med = statistics.median(sl)
        iqr = sl[(3 * len(sl)) // 4] - sl[len(sl) // 4]
        if med > 0 and iqr / med < 0.08:
            break
        add_pair(len(pairs))
    per_exec = statistics.median(slopes_of())
    slopes = slopes_of()
    return per_exec, {
        "batch": batch,
        "lo_reps": lo_reps,
        "timing_reps": timing_reps,
        "t1_totals_ms": [p[0] * 1e3 for p in pairs],
        "tR_totals_ms": [p[1] * 1e3 for p in pairs],
        "slopes_us": [s * 1e6 for s in slopes],
    }


def kernel(x, rotary_emb, w_qkv, w_out):
    in_maps = host_prep(x, rotary_emb, w_qkv, w_out)
    res = run_on_hw(in_maps)
    y = np.empty((B, N, H), dtype=np.float32)
    for b in range(B):
        y[b] = res.results[2 * b]["y"] + res.results[2 * b + 1]["y"]
    return y

